# revision 1
# baseline (speedup 1.0000x reference)
"""KAN (Jacobi/shared) kernel for Trainium2, 8 NeuronCores.

Math: y[b,o,s] = sum_{i,d} P_d(tanh(x[b,i,s])) * C[i,o,d],  P_d = Jacobi(a=1,b=1)
Monomial reformulation (host-side basis change, exact):
  P0=1; P1=2t; P2=3.75t^2-0.75; P3=7t^3-3t; P4=13.125t^4-8.75t^2+0.625
  => y[b,o,s] = bias[o] + sum_{k=1..4} sum_i t^k[b,i,s] * W[i,o,k]
Device: tanh on ScalarE, t^2/t^3/t^4 on VectorE, 4 accumulating K=64 matmuls
into PSUM on TensorE, bias folded into the PSUM->SBUF copy.

Sharding: split the 65536-point axis into 8 chunks of 8192 (data parallel),
replicate the tiny weights. Full inputs in, full output out.
"""

import sys

import numpy as np

if "/opt/trn_rl_repo" not in sys.path:
    sys.path.insert(0, "/opt/trn_rl_repo")

B = 4
I = 64
S = 65536
O = 128
NCORES = 8
SC = S // NCORES  # 8192 points per core
T = 512           # tile free dim (== one fp32 PSUM bank)
NJ = SC // T      # 16 column tiles per partition block
NP = (B * I) // 128  # 2 partition blocks (2 batches each)

# coeff of t^k (rows) in Jacobi P^(1,1)_d (cols), d=0..4
_MONO = np.array(
    [
        [1.0, 0.0, -0.75, 0.0, 0.625],
        [0.0, 2.0, 0.0, -3.0, 0.0],
        [0.0, 0.0, 3.75, 0.0, -8.75],
        [0.0, 0.0, 0.0, 7.0, 0.0],
        [0.0, 0.0, 0.0, 0.0, 13.125],
    ],
    dtype=np.float64,
)

MATMUL_DTYPE = "float32r"  # "float32" (exact, 4 cyc/row) or "float32r" (1 cyc/row)

# pool buffer counts (tunable)
BUFS = {"xin": 6, "t": 3, "t2": 3, "t3": 3, "t4": 3, "out": 6, "psum": 6}

_CACHE = {}


def _build_nc():
    import concourse.bacc as bacc
    import concourse.bass as bass
    import concourse.tile as tile
    from concourse import mybir

    f32 = mybir.dt.float32
    mm_dt = getattr(mybir.dt, MATMUL_DTYPE)

    nc = bacc.Bacc("TRN2", target_bir_lowering=False, debug=False)

    x_dram = nc.dram_tensor("x", [B * I, SC], f32, kind="ExternalInput")
    # w layout: [i, k, o] so each W_k slice is contiguous per partition
    w_dram = nc.dram_tensor("w", [I, 4, O], mm_dt, kind="ExternalInput")
    b_dram = nc.dram_tensor("bias", [O, 1], f32, kind="ExternalInput")
    y_dram = nc.dram_tensor("y", [B, O, SC], f32, kind="ExternalOutput")

    with tile.TileContext(nc) as tc:
        with (
            tc.tile_pool(name="consts", bufs=1) as consts,
            tc.tile_pool(name="xin", bufs=BUFS["xin"]) as xin_pool,
            tc.tile_pool(name="pt", bufs=BUFS["t"]) as t_pool,
            tc.tile_pool(name="pt2", bufs=BUFS["t2"]) as t2_pool,
            tc.tile_pool(name="pt3", bufs=BUFS["t3"]) as t3_pool,
            tc.tile_pool(name="pt4", bufs=BUFS["t4"]) as t4_pool,
            tc.tile_pool(name="out", bufs=BUFS["out"]) as out_pool,
            tc.tile_pool(name="psum", bufs=BUFS["psum"], space="PSUM") as psum_pool,
        ):
            # weights duplicated into both partition halves so lhsT/rhs base
            # partitions match for the upper-half (batch-odd) matmuls
            w_sb = consts.tile([128, 4, O], mm_dt)
            nc.sync.dma_start(out=w_sb[0:I, :, :], in_=w_dram[:, :, :])
            nc.sync.dma_start(out=w_sb[I:128, :, :], in_=w_dram[:, :, :])
            bias_sb = consts.tile([O, 1], f32)
            nc.sync.dma_start(out=bias_sb[:, :], in_=b_dram[:, :])

            xv = x_dram.ap()  # [256, SC]
            for p in range(NP):
                for j in range(NJ):
                    xin = xin_pool.tile([128, T], f32)
                    nc.sync.dma_start(
                        out=xin[:, :],
                        in_=xv[128 * p : 128 * (p + 1), T * j : T * (j + 1)],
                    )
                    t1 = t_pool.tile([128, T], mm_dt)
                    nc.scalar.activation(
                        t1[:, :], xin[:, :], mybir.ActivationFunctionType.Tanh
                    )
                    t2 = t2_pool.tile([128, T], mm_dt)
                    nc.vector.tensor_mul(t2[:, :], t1[:, :], t1[:, :])
                    t3 = t3_pool.tile([128, T], mm_dt)
                    nc.vector.tensor_mul(t3[:, :], t2[:, :], t1[:, :])
                    t4 = t4_pool.tile([128, T], mm_dt)
                    nc.vector.tensor_mul(t4[:, :], t2[:, :], t2[:, :])
                    pows = [t1, t2, t3, t4]
                    for h in range(2):
                        lo, hi = I * h, I * (h + 1)
                        ps = psum_pool.tile([O, T], f32)
                        for k in range(4):
                            nc.tensor.matmul(
                                ps[:, :],
                                w_sb[lo:hi, k, :],
                                pows[k][lo:hi, :],
                                start=(k == 0),
                                stop=(k == 3),
                            )
                        ot = out_pool.tile([O, T], f32)
                        if h == 0:
                            nc.scalar.activation(
                                ot[:, :],
                                ps[:, :],
                                mybir.ActivationFunctionType.Identity,
                                bias=bias_sb[:, 0:1],
                            )
                        else:
                            nc.vector.tensor_scalar_add(
                                ot[:, :], ps[:, :], bias_sb[:, 0:1]
                            )
                        nc.sync.dma_start(
                            out=y_dram[2 * p + h, :, T * j : T * (j + 1)],
                            in_=ot[:, :],
                        )
    nc.compile()
    return nc


def _get_nc():
    if "nc" not in _CACHE:
        _CACHE["nc"] = _build_nc()
    return _CACHE["nc"]


def _host_weights(jacobi_coeffs: np.ndarray):
    c = jacobi_coeffs.astype(np.float64)  # (I, O, 5)
    cm = np.einsum("iod,kd->iok", c, _MONO)  # monomial coords, k=0..4
    bias = cm[:, :, 0].sum(axis=0).astype(np.float32).reshape(O, 1)
    w = np.ascontiguousarray(
        cm[:, :, 1:].transpose(0, 2, 1).astype(np.float32)
    )  # (I, 4, O)
    return w, bias


def kernel(x: np.ndarray, jacobi_coeffs: np.ndarray) -> np.ndarray:
    from concourse.bass_utils import run_bass_kernel_spmd

    w, bias = _host_weights(np.asarray(jacobi_coeffs))
    x = np.asarray(x, dtype=np.float32)

    in_maps = []
    for c in range(NCORES):
        xc = np.ascontiguousarray(x[:, :, c * SC : (c + 1) * SC]).reshape(B * I, SC)
        in_maps.append({"x": xc, "w": w, "bias": bias})

    res = run_bass_kernel_spmd(_get_nc(), in_maps, core_ids=list(range(NCORES)))
    y = np.concatenate([r["y"] for r in res.results], axis=2)
    return np.ascontiguousarray(y, dtype=np.float32)

